# revision 1
# baseline (speedup 1.0000x reference)
"""Multi-head attention + residual + LayerNorm on 8 Trainium2 cores.

Model (per reference):
  Q/K/V = x @ W.T  (torch Linear), 16 heads x d_k=64, softmax(QK^T/8) @ V,
  out-proj, residual with query, LayerNorm.

Sharding: core c = (batch b = c//4, head-group hg = c%4 of 4 heads).
Launch 1 (per core): QKV projections for its 4 heads on its batch,
  attention scores^T = K @ Q^T (k on partitions), exp (no max-subtraction:
  scores ~ N(0,1), safe in fp32), context^T accumulated with a ones-column
  appended to V so the softmax denominator falls out of the same matmuls.
  Outputs ctx^T (unnormalized) + denominators.
Launch 2 (per core): 512 rows of B*S=4096: normalize ctx by denominators,
  out-projection, residual add, LayerNorm.

bf16 operands / fp32 PSUM accumulation throughout the attention path; the
fp32 residual (query passes through untouched) keeps final error ~1e-4.
"""

from contextlib import ExitStack

import numpy as np
import ml_dtypes

import concourse.bass as bass
import concourse.bacc as bacc
import concourse.tile as tile
from concourse import mybir
from concourse.bass_utils import run_bass_kernel_spmd

BF16 = mybir.dt.bfloat16
F32 = mybir.dt.float32
FP8 = mybir.dt.float8e4
NPBF16 = ml_dtypes.bfloat16
NPFP8 = ml_dtypes.float8_e4m3
QK_FP8 = True  # fp8 Q/K projection inputs (halves the DMA lead-in)
XDT, NPXDT = (FP8, NPFP8) if QK_FP8 else (BF16, NPBF16)

B, S, D = 2, 2048, 1024
H = 16
DK = 64
N_CORES = 8
H_LOC = 4          # heads per core
HP_LOC = 2         # head pairs per core
NCH = S // 128     # 16 k-chunks
NIC = D // 128     # 8 contraction chunks
SLOT_MM = 3        # 512-col matmuls per psum slot -> [128, 1536]
KQ = 4             # chunks per ctx sweep quarter
EPS = 1e-5

_cache = {}

E16_HOST = np.zeros((H, NIC * 128), NPBF16)
for _ic in range(NIC):
    for _j in range(2):
        E16_HOST[2 * _ic + _j, 128 * _ic + 64 * _j : 128 * _ic + 64 * _j + 64] = 1.0


def _f32r(ap):
    return ap.bitcast(mybir.dt.float32r)


def build_kernel1():
    nc = bacc.Bacc("TRN2", target_bir_lowering=False, debug=False)

    xq = nc.dram_tensor("xq", [D, S], XDT, kind="ExternalInput")
    xk = nc.dram_tensor("xk", [D, S], XDT, kind="ExternalInput")
    xv = nc.dram_tensor("xv", [D, S], BF16, kind="ExternalInput")
    wq = nc.dram_tensor("wq", [D, 256], XDT, kind="ExternalInput")
    wk = nc.dram_tensor("wk", [D, 256], XDT, kind="ExternalInput")
    wv = nc.dram_tensor("wv", [D, 256], BF16, kind="ExternalInput")
    ctxT = nc.dram_tensor("ctxT", [256, S], BF16, kind="ExternalOutput")
    dnm = nc.dram_tensor("dnm", [H_LOC, S], BF16, kind="ExternalOutput")

    with tile.TileContext(nc) as tc:
        with (
            tc.tile_pool(name="wp", bufs=1) as wp,
            tc.tile_pool(name="qk", bufs=1) as qkp,
            tc.tile_pool(name="va", bufs=1) as vap,
            tc.tile_pool(name="xv_p", bufs=1) as xvp,
            tc.tile_pool(name="ps", bufs=2, space="PSUM") as psp,
            tc.tile_pool(name="pc", bufs=1, space="PSUM") as pcp,
        ):
            qt = qkp.tile([128, HP_LOC, S], BF16)
            kt = qkp.tile([128, HP_LOC, S], BF16)
            vaug = vap.tile([128, HP_LOC, NCH, 130], BF16)
            w_qk = wp.tile([128, 2, NIC, 256], XDT)
            w_v = wp.tile([128, NIC, 256], BF16)
            x_v = xvp.tile([128, NIC, S], BF16)

            # ---- DMA emission order: q/k weights, q/k inputs (interleaved),
            # then v weights/inputs; lets QK-proj and attention start early.
            for ic in range(NIC):
                nc.sync.dma_start(out=w_qk[:, 0, ic, :], in_=wq[128 * ic : 128 * ic + 128, :])
                nc.sync.dma_start(out=w_qk[:, 1, ic, :], in_=wk[128 * ic : 128 * ic + 128, :])

            xqk_ctx = ExitStack()
            if True:
                xqkp = xqk_ctx.enter_context(tc.tile_pool(name="xqk", bufs=1))
                x_q = xqkp.tile([128, NIC, S], XDT)
                x_k = xqkp.tile([128, NIC, S], XDT)
                for ic in range(NIC):
                    nc.sync.dma_start(out=x_q[:, ic, :], in_=xq[128 * ic : 128 * ic + 128, :])
                    nc.sync.dma_start(out=x_k[:, ic, :], in_=xk[128 * ic : 128 * ic + 128, :])
                for ic in range(NIC):
                    nc.sync.dma_start(out=w_v[:, ic, :], in_=wv[128 * ic : 128 * ic + 128, :])
                for ic in range(NIC):
                    nc.sync.dma_start(out=x_v[:, ic, :], in_=xv[128 * ic : 128 * ic + 128, :])

                # ---- Q^T / K^T projections as deferrable groups: hp0's are
                # emitted immediately; hp1's are drip-fed into hp0's
                # ACT-bound attention stream so scores start ASAP.
                def proj_group(hp, cb, t):
                    xt, outt = ((x_q, qt), (x_k, kt))[t]
                    pt = psp.tile([128, 1024], F32, tag="slot")
                    for ic in range(NIC):
                        for j in range(2):
                            nc.tensor.matmul(
                                pt[:, 512 * j : 512 * j + 512],
                                w_qk[:, t, ic, 128 * hp : 128 * hp + 128],
                                xt[:, ic, 1024 * cb + 512 * j : 1024 * cb + 512 * j + 512],
                                start=(ic == 0),
                                stop=(ic == NIC - 1),
                            )
                    nc.vector.tensor_copy(
                        out=outt[:, hp, 1024 * cb : 1024 * cb + 1024], in_=pt[:]
                    )

                for cb in range(2):
                    for t in range(2):
                        proj_group(0, cb, t)
                hp1_groups = [(1, cb, t) for cb in range(2) for t in range(2)]

            # ---- attention (V projection interleaved into hp0's stream) ----
            attn_ctx = ExitStack()
            sxp = attn_ctx.enter_context(tc.tile_pool(name="sx", bufs=14))
            stp = attn_ctx.enter_context(tc.tile_pool(name="st", bufs=4))
            finp = attn_ctx.enter_context(tc.tile_pool(name="fin", bufs=2))
            # scores^T chunk: out [k=128, q-cols]; lhsT = K^T slice [64, 128] at
            # partition base 64*h, rhs = Q^T slice [64, q] same base -> natural
            # row-packed pairs (concurrent on PE).
            # exp via ScalarE straight from PSUM slots [128, 1536]; bf16 out.
            # ctx^T: [65, q] += Vaug(c,h).T @ expS(c) ; row 64 = denominator.
            nc.vector.memset(vaug[:], 1.0)

            def v_proj(c):
                # V in natural layout [s-block=c, d] + split into vaug slots
                pv = pcp.tile([128, 256], F32, tag="ctx")
                for ic in range(NIC):
                    nc.tensor.matmul(
                        pv[:],
                        x_v[:, ic, 128 * c : 128 * c + 128],
                        w_v[:, ic, :],
                        start=(ic == 0),
                        stop=(ic == NIC - 1),
                    )
                for hp2 in range(HP_LOC):
                    nc.vector.tensor_copy(
                        out=vaug[:, hp2, c, 0:130].rearrange(
                            "p (two f) -> p two f", two=2
                        )[:, :, 0:64],
                        in_=pv[:, 128 * hp2 : 128 * hp2 + 128].rearrange(
                            "p (two f) -> p two f", two=2
                        ),
                    )

            for hp in range(HP_LOC):
                # stream of scores mms for this head-pair, filling psum slots
                slot = None
                slot_fill = 0
                sx_map = {}  # (h, c, qq) -> (sx_tile, pos)
                pend = []
                ctx_sb = {}  # (h, qh) -> sbuf partial tile

                def flush_slot():
                    nonlocal slot, slot_fill
                    if slot is None or slot_fill == 0:
                        return
                    sx = sxp.tile([128, SLOT_MM * 512], BF16, tag="sx")
                    nc.scalar.activation(
                        out=sx[:, 0 : slot_fill * 512],
                        in_=slot[:, 0 : slot_fill * 512],
                        func=mybir.ActivationFunctionType.Exp,
                        scale=0.125,
                    )
                    for key, pos in pend:
                        sx_map[key] = (sx, pos)
                    pend.clear()
                    slot = None
                    slot_fill = 0

                def ctx_sweep(kq):
                    for h in range(2):
                        for qh in range(2):
                            pctx = pcp.tile([65, 1024], F32, tag="ctx")
                            for ci in range(KQ):
                                c = kq * KQ + ci
                                for qq in (2 * qh, 2 * qh + 1):
                                    sx, pos = sx_map.pop((h, c, qq))
                                    nc.tensor.matmul(
                                        pctx[:, 512 * (qq - 2 * qh) : 512 * (qq - 2 * qh) + 512],
                                        vaug[:, hp, c, 65 * h : 65 * h + 65],
                                        sx[:, 512 * pos : 512 * pos + 512],
                                        start=(ci == 0),
                                        stop=(ci == KQ - 1),
                                    )
                            part = ctx_sb.get((h, qh))
                            if part is None:
                                part = stp.tile([65, 1024], F32, tag="part")
                                ctx_sb[(h, qh)] = part
                                nc.vector.tensor_copy(out=part[:], in_=pctx[:])
                            elif kq < 3:
                                nc.vector.tensor_add(out=part[:], in0=part[:], in1=pctx[:])
                            else:
                                # final add incl. denominator row, then DMA out
                                hh = 2 * hp + h
                                fin = finp.tile([65, 1024], BF16, tag="fin")
                                nc.vector.tensor_add(out=fin[:], in0=part[:], in1=pctx[:])
                                nc.sync.dma_start(
                                    out=ctxT[64 * hh : 64 * hh + 64,
                                             1024 * qh : 1024 * qh + 1024],
                                    in_=fin[0:64, :],
                                )
                                nc.sync.dma_start(
                                    out=dnm[hh : hh + 1, 1024 * qh : 1024 * qh + 1024],
                                    in_=fin[64:65, :],
                                )
                                del ctx_sb[(h, qh)]

                for kq in range(4):
                    if kq > 0:
                        ctx_sweep(kq - 1)
                    for ci in range(KQ):
                        c = kq * KQ + ci
                        for qq in range(4):
                            for h in range(2):
                                if slot is None:
                                    slot = psp.tile([128, SLOT_MM * 512], F32, tag="slot")
                                    slot_fill = 0
                                nc.tensor.matmul(
                                    slot[:, 512 * slot_fill : 512 * slot_fill + 512],
                                    kt[64 * h : 64 * h + 64, hp, 128 * c : 128 * c + 128],
                                    qt[64 * h : 64 * h + 64, hp, 512 * qq : 512 * qq + 512],
                                    start=True,
                                    stop=True,
                                )
                                pend.append(((h, c, qq), slot_fill))
                                slot_fill += 1
                                if slot_fill == SLOT_MM:
                                    flush_slot()
                    flush_slot()  # kq boundary
                    if hp == 0:
                        # drip-feed V projection + hp1 QK projection into the
                        # ACT-bound stream
                        for ci in range(KQ):
                            v_proj(kq * KQ + ci)
                        if kq in (1, 2):
                            proj_group(*hp1_groups[2 * (kq - 1)])
                            proj_group(*hp1_groups[2 * (kq - 1) + 1])
                ctx_sweep(3)
            attn_ctx.close()
            xqk_ctx.close()

    nc.compile()
    return nc


def build_kernel2():
    nc = bacc.Bacc("TRN2", target_bir_lowering=False, debug=False)

    R = 512  # rows per core
    ctxT = nc.dram_tensor("ctxT", [D, R], BF16, kind="ExternalInput")
    dnm = nc.dram_tensor("dnm", [H, R], BF16, kind="ExternalInput")
    woT = nc.dram_tensor("woT", [D, D], BF16, kind="ExternalInput")
    xres = nc.dram_tensor("xres", [R, D], F32, kind="ExternalInput")
    e16d = nc.dram_tensor("e16", [H, NIC * 128], BF16, kind="ExternalInput")
    gamma = nc.dram_tensor("gamma", [1, D], F32, kind="ExternalInput")
    beta = nc.dram_tensor("beta", [1, D], F32, kind="ExternalInput")
    out = nc.dram_tensor("out", [R, D], F32, kind="ExternalOutput")

    with tile.TileContext(nc) as tc:
        with (
            tc.tile_pool(name="wo", bufs=1) as wop,
            tc.tile_pool(name="cx", bufs=1) as cxp,
            tc.tile_pool(name="sm", bufs=1) as smp,
            tc.tile_pool(name="wk", bufs=3) as wkp,
            tc.tile_pool(name="ps", bufs=2, space="PSUM") as psp,
            tc.tile_pool(name="pb", bufs=2, space="PSUM") as pbp,
        ):
            wo_t = wop.tile([128, NIC, D], BF16)
            ctx_t = cxp.tile([128, NIC, R], BF16)
            dnm_t = smp.tile([H, R], BF16)
            nc.sync.dma_start(out=dnm_t[:], in_=dnm[:])
            for ic in range(NIC):
                nc.sync.dma_start(out=ctx_t[:, ic, :], in_=ctxT[128 * ic : 128 * ic + 128, :])
                nc.sync.dma_start(out=wo_t[:, ic, :], in_=woT[128 * ic : 128 * ic + 128, :])
            rec_t = smp.tile([H, R], BF16)
            with nc.allow_low_precision(reason="softmax denom recip in bf16; residual dominates output"):
                nc.vector.reciprocal(out=rec_t[:], in_=dnm_t[:])

            # E16 selection matrices: E16[h, ic, j] = 1 iff h == 2*ic + j//64
            e16 = smp.tile([H, NIC, 128], BF16)
            nc.sync.dma_start(out=e16[:].rearrange("h a b -> h (a b)"), in_=e16d[:])

            gb = smp.tile([128, D], F32)
            bb = smp.tile([128, D], F32)
            g_ap = gamma.ap()
            b_ap = beta.ap()
            nc.sync.dma_start(
                out=gb[:], in_=bass.AP(tensor=g_ap.tensor, offset=g_ap.offset,
                                       ap=[[0, 128], [1, D]])
            )
            nc.sync.dma_start(
                out=bb[:], in_=bass.AP(tensor=b_ap.tensor, offset=b_ap.offset,
                                       ap=[[0, 128], [1, D]])
            )
            eps_t = smp.tile([128, 1], F32)
            nc.vector.memset(eps_t[:], EPS)

            # normalize ctx^T by per-(head, row) denominators -> bf16 lhsT tiles
            ctxn = cxp.tile([128, NIC, R], BF16)
            for ic in range(NIC):
                pb = pbp.tile([128, R], F32, tag="pb")
                nc.tensor.matmul(
                    pb[:], e16[:, ic, :], rec_t[:],
                    start=True, stop=True,
                )
                nc.vector.tensor_mul(out=ctxn[:, ic, :], in0=ctx_t[:, ic, :], in1=pb[:])

            # out-projection + residual + LayerNorm, 128 rows at a time
            for sc in range(4):
                po = psp.tile([128, D], F32, tag="po")
                for ic in range(NIC):
                    for j in range(2):
                        nc.tensor.matmul(
                            po[:, 512 * j : 512 * j + 512],
                            ctxn[:, ic, 128 * sc : 128 * sc + 128],
                            wo_t[:, ic, 512 * j : 512 * j + 512],
                            start=(ic == 0),
                            stop=(ic == NIC - 1),
                        )
                xq_sb = wkp.tile([128, D], F32, tag="xq")
                nc.sync.dma_start(out=xq_sb[:], in_=xres[128 * sc : 128 * sc + 128, :])
                x_sb = wkp.tile([128, D], F32, tag="x")
                nc.vector.tensor_add(out=x_sb[:], in0=po[:], in1=xq_sb[:])

                stats = wkp.tile([128, 2, 6], F32, tag="bn")
                for g in range(2):
                    nc.vector.bn_stats(out=stats[:, g, :], in_=x_sb[:, 512 * g : 512 * g + 512])
                mv = wkp.tile([128, 2], F32, tag="mv")
                nc.vector.bn_aggr(out=mv[:], in_=stats[:])
                std = wkp.tile([128, 1], F32, tag="std")
                nc.scalar.activation(
                    out=std[:], in_=mv[:, 1:2],
                    func=mybir.ActivationFunctionType.Sqrt,
                    bias=eps_t[:], scale=1.0,
                )
                rstd = wkp.tile([128, 1], F32, tag="rstd")
                nc.vector.reciprocal(out=rstd[:], in_=std[:])
                xn = wkp.tile([128, D], F32, tag="xn")
                nc.vector.tensor_scalar(
                    out=xn[:], in0=x_sb[:],
                    scalar1=mv[:, 0:1], scalar2=rstd[:],
                    op0=mybir.AluOpType.subtract, op1=mybir.AluOpType.mult,
                )
                xg = wkp.tile([128, D], F32, tag="xg")
                nc.vector.tensor_mul(out=xg[:], in0=xn[:], in1=gb[:])
                xb = wkp.tile([128, D], F32, tag="xb")
                nc.vector.tensor_add(out=xb[:], in0=xg[:], in1=bb[:])
                nc.sync.dma_start(out=out[128 * sc : 128 * sc + 128, :], in_=xb[:])

    nc.compile()
    return nc


def _get(name):
    if name not in _cache:
        _cache[name] = build_kernel1() if name == "k1" else build_kernel2()
    return _cache[name]


def kernel(query, key, value, w_q, w_k, w_v, w_o, ln_gamma, ln_beta):
    query = np.asarray(query, np.float32)
    key = np.asarray(key, np.float32)
    value = np.asarray(value, np.float32)
    w_q = np.asarray(w_q, np.float32)
    w_k = np.asarray(w_k, np.float32)
    w_v = np.asarray(w_v, np.float32)
    w_o = np.asarray(w_o, np.float32)
    ln_gamma = np.asarray(ln_gamma, np.float32)
    ln_beta = np.asarray(ln_beta, np.float32)

    nc1 = _get("k1")
    nc2 = _get("k2")

    xqT = [np.ascontiguousarray(query[b].T).astype(NPXDT) for b in range(B)]
    xkT = [np.ascontiguousarray(key[b].T).astype(NPXDT) for b in range(B)]
    xvT = [np.ascontiguousarray(value[b].T).astype(NPBF16) for b in range(B)]
    wqT = np.ascontiguousarray(w_q.T).astype(NPXDT)
    wkT = np.ascontiguousarray(w_k.T).astype(NPXDT)
    wvT = np.ascontiguousarray(w_v.T).astype(NPBF16)

    in_maps1 = []
    for c in range(N_CORES):
        b, hg = c // 4, c % 4
        in_maps1.append({
            "xq": xqT[b], "xk": xkT[b], "xv": xvT[b],
            "wq": np.ascontiguousarray(wqT[:, 256 * hg : 256 * hg + 256]),
            "wk": np.ascontiguousarray(wkT[:, 256 * hg : 256 * hg + 256]),
            "wv": np.ascontiguousarray(wvT[:, 256 * hg : 256 * hg + 256]),
        })
    res1 = run_bass_kernel_spmd(nc1, in_maps1, core_ids=list(range(N_CORES)))

    ctxT_full = np.empty((D, B * S), NPBF16)
    dnm_full = np.empty((H, B * S), NPBF16)
    for c in range(N_CORES):
        b, hg = c // 4, c % 4
        ctxT_full[256 * hg : 256 * hg + 256, S * b : S * b + S] = res1.results[c]["ctxT"]
        dnm_full[4 * hg : 4 * hg + 4, S * b : S * b + S] = res1.results[c]["dnm"]

    woT = np.ascontiguousarray(w_o.T).astype(NPBF16)
    q_flat = query.reshape(B * S, D)
    g2 = ln_gamma.reshape(1, D)
    b2 = ln_beta.reshape(1, D)

    in_maps2 = []
    for c in range(N_CORES):
        r0 = 512 * c
        in_maps2.append({
            "ctxT": np.ascontiguousarray(ctxT_full[:, r0 : r0 + 512]),
            "dnm": np.ascontiguousarray(dnm_full[:, r0 : r0 + 512]),
            "woT": woT,
            "xres": np.ascontiguousarray(q_flat[r0 : r0 + 512, :]),
            "e16": E16_HOST,
            "gamma": g2, "beta": b2,
        })
    res2 = run_bass_kernel_spmd(nc2, in_maps2, core_ids=list(range(N_CORES)))

    out = np.concatenate([res2.results[c]["out"] for c in range(N_CORES)], axis=0)
    return out.reshape(B, S, D)

